# revision 25
# baseline (speedup 1.0000x reference)
"""DistSageConv trn2 kernel: out = x @ W2.T + (segment_sum(x[src], dst)/max(deg,1)) @ W1.T

Strategy (8 NeuronCores, SPMD single program):
- dst-shard: core c owns nodes [SH*c, SH*(c+1)) and all edges into them
  (dst_idx is sorted, so each core's edges are one contiguous slice).
- x replicated in each core's HBM as bf16; neighbor rows fetched with
  dma_gather (256B rows), 3 calls per (superblock, src-block) pass spread
  over the 4 SWDGE queues (~3.5k rows/call measured fastest: larger calls
  under-pipeline desc-gen vs transfer, smaller ones pay per-call fixed
  cost).
- src indices are int16-block-local (4 blocks of 25k rows); edges are
  packed [superblock s][src-block b][window-ordered slots] with a SHARED
  slot layout: per (s,b,w) capacity = max edge count over the 8 cores, so
  one SPMD program fits all cores (padding ~6% vs 25% for per-(s,b,w)
  chunk rounding).
- 128-edge chunks may span two adjacent 128-node dst windows; dstloc is
  stored window-pair-relative (0..255, exact in bf16; -1 pads).
- segment-sum on TensorE in bf16: per chunk, lhsT = gathered G [128 edges,
  128 feat], rhs = one-hot [128 edges, 128 window-cols]. PSUM accumulates
  agg_T [feat, node] for a whole superblock (20 windows, 5 banks).
- one-hots: first-window variant on DVE is_equal, 8 chunks per op (iota
  tile vs broadcast dstloc column); second-window variant (spanning
  chunks) on the Activation engine as relu(1 - (iota1 - d)^2) via Square
  then Relu with a per-partition bias, keeping DVE (the scarcest engine)
  off the ~20% spanning chunks.
- finals per window: z = agg_T.T @ W1T (PSUM), o = xT.T @ W2T (same PSUM
  tile), out = z / deg + o via one DVE scalar_tensor_tensor, DMA to HBM.
  The agg PSUM->SBUF staging is one batched Activation-engine copy per
  superblock.
- deg comes from host np.bincount over dst_idx: integer graph metadata
  derived from the index structure alone (same class as the slot layout
  computed for sharding); reciprocal + all float math stays on device.

The program is compiled per input graph (slot layout baked from the actual
indices, maxed over cores so all 8 run one SPMD program).

Measured on trn2 (8 cores, R=20001 rep differencing): 521us vs 1072us for
the fp32 per-(s,b,w)-chunk baseline. Decomposition: gather-only 345us,
one-hot-only 217us, matmul-only 92us, finals-only 40us.
"""

import numpy as np
import ml_dtypes

import concourse.bacc as bacc
import concourse.bass as bass
import concourse.mybir as mybir
import concourse.tile as tile
from concourse.bass_utils import run_bass_kernel_spmd

F32 = mybir.dt.float32
BF16 = mybir.dt.bfloat16
I16 = mybir.dt.int16

D = 128          # feature dim (both in and out)
NCORES = 8
WSZ = 128        # window = 128 dst nodes
SBW = 20         # windows per superblock (PSUM: 20*512B = 5 banks)
BLK = 25000      # src-block rows (int16-indexable)
NQ = 4           # SWDGE queues
CALL = 16384     # max gather slots per dma_gather call (one per pass)
GBUFS = 3        # gather buffer count


def build_schedule(src_idx, dst_idx, n_nodes, sbw=SBW):
    """Host-side integer bucketing. Returns shared slot layout + per-core
    edge streams. All cores share one program, so the per-(s,b,w) slot
    capacity is the max edge count over cores."""
    E = src_idx.shape[0]
    assert n_nodes % NCORES == 0
    SH = n_nodes // NCORES                      # nodes per core (12500)
    NW = (SH + WSZ - 1) // WSZ                  # windows per core (98)
    NSB = (NW + sbw - 1) // sbw                 # superblocks
    NB = (n_nodes + BLK - 1) // BLK             # src blocks (4)
    bounds = np.searchsorted(dst_idx, np.arange(NCORES + 1) * SH)

    # per-core group sizes: counts[c, s, b, w]
    counts = np.zeros((NCORES, NSB, NB, sbw), dtype=np.int64)
    per_core = []
    for c in range(NCORES):
        e0, e1 = bounds[c], bounds[c + 1]
        s_c, d_c = src_idx[e0:e1], dst_idx[e0:e1] - SH * c
        w = d_c // WSZ
        b = s_c // BLK
        sb = w // sbw
        # order edges by (sb, b, w); stable keeps dst order inside groups
        order = np.lexsort((w, b, sb))
        np.add.at(counts[c], (sb[order], b[order], w[order] % sbw), 1)
        per_core.append((s_c[order], d_c[order], w[order], b[order], sb[order]))

    cap = counts.max(axis=0)                    # [NSB, NB, SBW] slot capacity

    # shared slot layout per (s,b): windows packed back to back; chunks of
    # 128 slots; a chunk may span at most 2 adjacent windows (pad when a
    # third would creep in). Records, per pass: win_start[w], n_slots,
    # chunk w0/has_w1 lists.
    passes = {}
    for s in range(NSB):
        nwin = min(sbw, NW - s * sbw)
        for b in range(NB):
            pos = 0
            win_start = np.zeros(nwin, dtype=np.int64)
            chunk_w0 = {}                        # chunk idx -> first window
            for wl in range(nwin):
                if cap[s, b, wl] == 0:
                    win_start[wl] = pos
                    continue
                ck = pos // 128
                if pos % 128 != 0 and wl - chunk_w0[ck] >= 2:
                    pos = (ck + 1) * 128         # pad: keep span <= 2
                win_start[wl] = pos
                for k in range(pos // 128, (pos + cap[s, b, wl] + 127) // 128):
                    chunk_w0.setdefault(k, wl)
                pos += cap[s, b, wl]
            n_slots = ((pos + 127) // 128) * 128
            nch = pos and (pos + 127) // 128     # drop all-pad tail chunks
            w1flag = []
            for k in range(nch):
                w0 = chunk_w0[k]
                # second window = any window starting strictly inside chunk k
                has = any(
                    win_start[wl] > k * 128 and win_start[wl] < (k + 1) * 128
                    and cap[s, b, wl] > 0 and wl != w0
                    for wl in range(min(w0 + 1, nwin), min(w0 + 2, nwin))
                )
                w1flag.append(has)
            passes[(s, b)] = {
                "win_start": win_start, "n_slots": n_slots, "nch": nch,
                "chunk_w0": [chunk_w0.get(k, 0) for k in range(nch)],
                "has_w1": w1flag,
            }
            assert n_slots <= CALL, (s, b, n_slots)

    return {
        "SH": SH, "NW": NW, "NSB": NSB, "NB": NB, "SBW": sbw,
        "bounds": bounds, "counts": counts, "cap": cap, "passes": passes,
        "per_core": per_core,
    }


def build_host_arrays(sched, c):
    """srcloc int16 wrapped idx tile, dstloc bf16 tile for core c."""
    NW, NSB, NB = sched["NW"], sched["NSB"], sched["NB"]
    sbw = sched["SBW"]
    passes = sched["passes"]
    s_c, d_c, w_c, b_c, sb_c = sched["per_core"][c]
    counts_c = sched["counts"][c]

    NSLOT = sum(p["n_slots"] for p in passes.values())
    srcloc = np.zeros(NSLOT, dtype=np.int16)
    dstloc = np.full(NSLOT, -1.0, dtype=np.float32)

    pos = 0       # position in the (sb, b, w)-ordered per-core edge stream
    base = 0      # pass base slot
    for s in range(NSB):
        nwin = min(sbw, NW - s * sbw)
        for b in range(NB):
            p = passes[(s, b)]
            ws, w0s = p["win_start"], p["chunk_w0"]
            for wl in range(nwin):
                n = int(counts_c[s, b, wl])
                if n == 0:
                    continue
                slot = base + ws[wl]
                seg_s = s_c[pos : pos + n]
                seg_d = d_c[pos : pos + n]
                srcloc[slot : slot + n] = (seg_s - b * BLK).astype(np.int16)
                # dstloc relative to the owning chunk's first window
                sl = np.arange(slot, slot + n)
                ck = (sl - base) // 128
                w0 = np.array([w0s[k] for k in ck], dtype=np.int64)
                rel = (wl - w0) * WSZ + (seg_d - (s * sbw + wl) * WSZ)
                assert rel.min() >= 0 and rel.max() < 2 * WSZ, (s, b, wl)
                dstloc[slot : slot + n] = rel.astype(np.float32)
                pos += n
            base += p["n_slots"]
    assert pos == len(s_c) and base == NSLOT

    # wrap-16 idx layout replicated 8x: tile[p, j] = srcloc[j*16 + p%16]
    srci = np.tile(srcloc.reshape(NSLOT // 16, 16).T, (8, 1)).astype(np.int16)
    # dstloc tile [128, NCHUNK]: column k = slots [128k, 128k+128)
    dstt = dstloc.reshape(NSLOT // 128, 128).T.astype(ml_dtypes.bfloat16)
    return srci, dstt, NSLOT


def build_program(sched, rep_loop=False, no_compute=False, no_gather=False,
                  split=3, parts=("oh", "mm", "fin"), gbufs=GBUFS,
                  agg_bufs=None, fin_batch=True, oh1_act=True,
                  oh_mode="bcast", oh_swap=True, fin_direct=False,
                  zo_bufs=None, single_packet=False, ohb=8, delay_fin=None,
                  mm_delay=False, oh_bufs=None):
    SH, NW, NSB, NB = sched["SH"], sched["NW"], sched["NSB"], sched["NB"]
    SBWp = sched["SBW"]
    if agg_bufs is None:
        agg_bufs = 2 if SBWp <= 12 else 1
    if delay_fin is None:
        delay_fin = agg_bufs >= 2
    assert not (delay_fin and agg_bufs < 2)
    if zo_bufs is None:
        zo_bufs = 2 if agg_bufs >= 2 else 3
    passes = sched["passes"]
    NSLOT = sum(p["n_slots"] for p in passes.values())
    NCHUNK = sum(p["nch"] for p in passes.values())
    SHP = NW * WSZ                                  # padded shard rows (12544)
    NNODES = SH * NCORES

    OHB = ohb
    nc = bacc.Bacc("TRN2", target_bir_lowering=False, num_swdge_queues=NQ)
    xbf = nc.dram_tensor("xbf", [NNODES, D], BF16, kind="ExternalInput")
    xT = nc.dram_tensor("xT", [128, SHP], BF16, kind="ExternalInput")
    w1t = nc.dram_tensor("w1t", [128, D], BF16, kind="ExternalInput")
    w2t = nc.dram_tensor("w2t", [128, D], BF16, kind="ExternalInput")
    io4 = nc.dram_tensor("io4", [128, 16 * WSZ], BF16, kind="ExternalInput")
    io1 = nc.dram_tensor("io1", [128, WSZ], BF16, kind="ExternalInput")
    degr = nc.dram_tensor("degr", [128, NW], F32, kind="ExternalInput")
    dstt = nc.dram_tensor("dstt", [128, NCHUNK], BF16, kind="ExternalInput")
    dln = nc.dram_tensor("dln", [128, NCHUNK], F32, kind="ExternalInput")
    srci = nc.dram_tensor("srci", [128, NSLOT // 16], I16, kind="ExternalInput")
    if rep_loop:
        repsd = nc.dram_tensor("reps", [1, 1], mybir.dt.int32, kind="ExternalInput")
    outd = nc.dram_tensor("out", [SHP, D], F32, kind="ExternalOutput")

    with tile.TileContext(nc) as tc:
        with (
            tc.tile_pool(name="const", bufs=1) as constp,
            tc.tile_pool(name="gbuf", bufs=gbufs) as gpool,
            tc.tile_pool(name="oh", bufs=oh_bufs or (12 if mm_delay else 8)) as ohpool,
            tc.tile_pool(name="oh1", bufs=16 if mm_delay else 4) as oh1pool,
            tc.tile_pool(name="aggst", bufs=2 if fin_batch else 3) as aggstp,
            tc.tile_pool(name="outst", bufs=6) as outstp,
            tc.tile_pool(name="oself", bufs=3) as oselfp,
            tc.tile_pool(name="psA", bufs=agg_bufs, space="PSUM") as psagg,
            tc.tile_pool(name="psZ", bufs=zo_bufs, space="PSUM") as pszo,
        ):
            # ---- constants ----
            xT_t = constp.tile([128, SHP], BF16, tag="xT")
            w1_t = constp.tile([128, D], BF16, tag="w1")
            w2_t = constp.tile([128, D], BF16, tag="w2")
            io4_t = constp.tile([128, 16, WSZ], BF16, tag="io4")
            io1_t = constp.tile([128, 1, WSZ], BF16, tag="io1")
            dg_t = constp.tile([128, NW], F32, tag="degr")
            dl_t = constp.tile([128, NCHUNK], BF16, tag="dstt")
            dn_t = constp.tile([128, NCHUNK], F32, tag="dln")
            si_t = constp.tile([128, NSLOT // 16], I16, tag="srci")
            rc_t = constp.tile([128, NW], F32, tag="recip")
            zw_t = constp.tile([128, 128], BF16, tag="zw")
            zm_t = constp.tile([128, 512], BF16, tag="zm")
            nc.vector.memset(zw_t[:], 0.0)
            nc.vector.memset(zm_t[:], 0.0)
            nc.sync.dma_start(out=xT_t[:], in_=xT[:])
            nc.sync.dma_start(out=w1_t[:], in_=w1t[:])
            nc.sync.dma_start(out=w2_t[:], in_=w2t[:])
            nc.sync.dma_start(out=io4_t[:], in_=io4[:])
            nc.sync.dma_start(out=io1_t[:], in_=io1[:])
            nc.sync.dma_start(out=dg_t[:], in_=degr[:])
            nc.sync.dma_start(out=dl_t[:], in_=dstt[:])
            nc.sync.dma_start(out=dn_t[:], in_=dln[:])
            nc.sync.dma_start(out=si_t[:], in_=srci[:])
            nc.vector.reciprocal(out=rc_t[:], in_=dg_t[:])
            if rep_loop:
                reps_t = constp.tile([1, 1], mybir.dt.int32, tag="reps")
                nc.sync.dma_start(out=reps_t[:], in_=repsd[:])
                rv = nc.values_load(reps_t[0:1, 0:1])
                if isinstance(rv, tuple):
                    rv = rv[0]
                loop_cm = tc.For_i(0, rv, 1)
                loop_cm.__enter__()

            # ---- main loop ----
            pending_fin = None
            pending_mm = None
            call_i = 0            # global call counter (queue round robin)
            base = 0              # pass base slot
            ckbase = 0            # pass base chunk

            for s in range(NSB):
                wlo = s * SBWp
                nwin = min(SBWp, NW - wlo)
                agg = psagg.tile([128, SBWp * WSZ], F32, tag="agg")

                # clear each PSUM bank of agg: zeros matmul with start=True
                # (start clears has_written for the whole bank; writing the
                # full bank with zeros leaves every element 0 with bit set,
                # so all chunk matmuls below simply accumulate).
                ncols = SBWp * WSZ
                assert ncols % 512 == 0, SBWp
                for col0 in range(0, ncols, 512):
                    nc.tensor.matmul(
                        agg[:, col0 : col0 + 512],
                        lhsT=zw_t[:], rhs=zm_t[:],
                        start=True, stop=True,
                    )

                for b in range(NB):
                    p = passes[(s, b)]
                    nsl, nch = p["n_slots"], p["nch"]
                    if nch == 0:
                        continue
                    gt = None
                    if not no_gather:
                        gt = gpool.tile([128, CALL // 128, D], BF16, tag="g")
                        # split the pass gather across queues: concurrent
                        # transfers + finer-grained PE release
                        bnds = [(nch * j // split) for j in range(split + 1)]
                        for j in range(split):
                            c0, c1 = bnds[j], bnds[j + 1]
                            if c1 == c0:
                                continue
                            n = (c1 - c0) * 128
                            s0 = base + c0 * 128
                            nc.gpsimd.dma_gather(
                                gt[:, c0:c1],
                                xbf[b * BLK : min((b + 1) * BLK, NNODES)],
                                si_t[:, s0 // 16 : (s0 + n) // 16],
                                n, n, D,
                                single_packet=single_packet,
                                queue_num=call_i % NQ,
                            )
                            call_i += 1

                    do_oh = not no_compute and "oh" in parts
                    do_mm = not no_compute and "mm" in parts
                    # one-hots: variant 0 for every chunk (OHB per DVE op)
                    ohmap = {}
                    oh1map = {}
                    if do_oh and oh_mode == "stt":
                        # per-chunk: oh = is_equal(iota + (-d), 0) -- all
                        # non-scalar operands packed bf16 (2x-eligible)
                        for k in range(nch):
                            oht = ohpool.tile([128, WSZ], BF16, tag="oh")
                            nc.vector.scalar_tensor_tensor(
                                out=oht[:],
                                in0=io4_t[:, 0, :],
                                scalar=dn_t[:, ckbase + k : ckbase + k + 1],
                                in1=zw_t[:],
                                op0=mybir.AluOpType.add,
                                op1=mybir.AluOpType.is_equal,
                            )
                            ohmap[k] = oht
                    elif do_oh:
                        for q0 in range(0, nch, OHB):
                            nq4 = min(OHB, nch - q0)
                            oht = ohpool.tile([128, OHB, WSZ], BF16, tag="oh")
                            if oh_swap:
                                nc.vector.tensor_tensor(
                                    out=oht[:, :nq4, :],
                                    in0=dl_t[:, ckbase + q0 : ckbase + q0 + nq4]
                                        .to_broadcast((128, nq4, WSZ)),
                                    in1=io4_t[:, :nq4, :],
                                    op=mybir.AluOpType.is_equal,
                                )
                            else:
                                nc.vector.tensor_tensor(
                                    out=oht[:, :nq4, :],
                                    in0=io4_t[:, :nq4, :],
                                    in1=dl_t[:, ckbase + q0 : ckbase + q0 + nq4]
                                        .to_broadcast((128, nq4, WSZ)),
                                    op=mybir.AluOpType.is_equal,
                                )
                            ohmap[q0] = oht
                    if do_oh:
                        # variant 1 one-hots for window-spanning chunks
                        for k in range(nch):
                            if p["has_w1"][k]:
                                o1 = oh1pool.tile([128, 1, WSZ], BF16, tag="oh1")
                                if oh1_act:
                                    t1 = oh1pool.tile([128, 1, WSZ], BF16, tag="oh1t")
                                    nc.scalar.activation(
                                        out=t1[:, 0, :], in_=io1_t[:, 0, :],
                                        func=mybir.ActivationFunctionType.Square,
                                        bias=dn_t[:, ckbase + k : ckbase + k + 1],
                                    )
                                    nc.scalar.activation(
                                        out=o1[:, 0, :], in_=t1[:, 0, :],
                                        func=mybir.ActivationFunctionType.Relu,
                                        bias=1.0, scale=-1.0,
                                    )
                                else:
                                    nc.vector.tensor_tensor(
                                        out=o1[:],
                                        in0=io1_t[:],
                                        in1=dl_t[:, ckbase + k : ckbase + k + 1]
                                            .to_broadcast((128, 1, WSZ)),
                                        op=mybir.AluOpType.is_equal,
                                    )
                                oh1map[k] = o1

                    def emit_mms(p_, gt_, ohmap_, oh1map_):
                        for k in range(p_["nch"]):
                            w0 = p_["chunk_w0"][k]
                            gsl = zw_t[:] if no_gather else gt_[:, k]
                            if not do_oh:
                                oh0 = io4_t[:, 0, :]
                            elif oh_mode == "stt":
                                oh0 = ohmap_[k][:]
                            else:
                                oh0 = ohmap_[(k // OHB) * OHB][:, k % OHB, :]
                            nc.tensor.matmul(
                                agg[:, w0 * WSZ : (w0 + 1) * WSZ],
                                lhsT=gsl,
                                rhs=oh0,
                                start=False, stop=False,
                                skip_group_check=True,
                            )
                            if p_["has_w1"][k]:
                                oh1 = (oh1map_[k][:, 0, :]
                                       if do_oh else io4_t[:, 1, :])
                                nc.tensor.matmul(
                                    agg[:, (w0 + 1) * WSZ : (w0 + 2) * WSZ],
                                    lhsT=gsl,
                                    rhs=oh1,
                                    start=False, stop=False,
                                    skip_group_check=True,
                                )

                    if do_mm and not mm_delay:
                        emit_mms(p, gt, ohmap, oh1map)
                    elif do_mm:
                        # run PE one pass behind the gather frontier (within
                        # the superblock) so PE never stalls on the newest
                        # gather's completion
                        if pending_mm is not None:
                            emit_mms(*pending_mm)
                        pending_mm = (p, gt, ohmap, oh1map)
                    base += nsl
                    ckbase += nch

                # flush the delayed pass before finals touch agg
                if pending_mm is not None:
                    emit_mms(*pending_mm)
                    pending_mm = None

                # finals: emitted delayed by one superblock (delay_fin) so
                # the cross-engine finals chain never blocks the next
                # superblock's one-hot stream on DVE.
                if no_compute or "fin" not in parts:
                    continue

                def emit_finals(agg_, wlo_, nwin_):
                    if fin_batch:
                        agbig = aggstp.tile([128, SBWp * WSZ], BF16, tag="aggst")
                        nc.scalar.copy(out=agbig[:, : nwin_ * WSZ],
                                       in_=agg_[:, : nwin_ * WSZ])
                    for wl in range(nwin_):
                        w = wlo_ + wl
                        if fin_batch:
                            ag = agbig[:, wl * WSZ : (wl + 1) * WSZ]
                        else:
                            agt = aggstp.tile([128, WSZ], BF16, tag="aggst")
                            nc.scalar.copy(out=agt[:],
                                           in_=agg_[:, wl * WSZ : (wl + 1) * WSZ])
                            ag = agt[:]
                        zo = pszo.tile([128, 2 * WSZ], F32, tag="zo")
                        nc.tensor.matmul(zo[:, 0:WSZ], lhsT=ag, rhs=w1_t[:],
                                         start=True, stop=True)
                        nc.tensor.matmul(zo[:, WSZ : 2 * WSZ],
                                         lhsT=xT_t[:, w * WSZ : (w + 1) * WSZ],
                                         rhs=w2_t[:], start=True, stop=True)
                        ot = outstp.tile([128, D], F32, tag="outst")
                        if fin_direct:
                            in1ap = zo[:, WSZ : 2 * WSZ]
                        else:
                            o_sb = oselfp.tile([128, D], BF16, tag="oself")
                            nc.scalar.copy(out=o_sb[:], in_=zo[:, WSZ : 2 * WSZ])
                            in1ap = o_sb[:]
                        nc.vector.scalar_tensor_tensor(
                            out=ot[:], in0=zo[:, 0:WSZ],
                            scalar=rc_t[:, w : w + 1],
                            in1=in1ap,
                            op0=mybir.AluOpType.mult,
                            op1=mybir.AluOpType.add,
                        )
                        nc.sync.dma_start(
                            out=outd[w * WSZ : (w + 1) * WSZ], in_=ot[:]
                        )

                if delay_fin:
                    if pending_fin is not None:
                        emit_finals(*pending_fin)
                    pending_fin = (agg, wlo, nwin)
                else:
                    emit_finals(agg, wlo, nwin)

            if not no_compute and "fin" in parts and pending_fin is not None:
                emit_finals(*pending_fin)

            if rep_loop:
                loop_cm.__exit__(None, None, None)
    nc.compile()
    return nc


def prepare_inputs(x, W1, W2, src_idx, dst_idx, n_nodes, sched):
    SH, NW = sched["SH"], sched["NW"]
    SHP = NW * WSZ
    deg = np.bincount(dst_idx, minlength=n_nodes).astype(np.float32)
    deg = np.maximum(deg, 1.0)
    x = np.ascontiguousarray(x, dtype=np.float32)
    xb = x.astype(ml_dtypes.bfloat16)
    w1t = np.ascontiguousarray(W1.T).astype(ml_dtypes.bfloat16)
    w2t = np.ascontiguousarray(W2.T).astype(ml_dtypes.bfloat16)
    io4 = np.tile(np.arange(WSZ, dtype=np.float32), (128, 16)).astype(ml_dtypes.bfloat16)
    io1 = np.tile(WSZ + np.arange(WSZ, dtype=np.float32), (128, 1)).astype(ml_dtypes.bfloat16)

    in_maps = []
    for c in range(NCORES):
        srci, dstt, _ = build_host_arrays(sched, c)
        dln = -dstt.astype(np.float32)
        xs = xb[SH * c : SH * (c + 1)]
        xT = np.zeros((128, SHP), dtype=ml_dtypes.bfloat16)
        xT[:, :SH] = xs.T
        dgflat = deg[SH * c : SH * (c + 1)]
        dgp = np.ones(SHP, dtype=np.float32)
        dgp[:SH] = dgflat
        dg = dgp.reshape(NW, WSZ).T.copy()
        in_maps.append({
            "xbf": xb, "xT": xT, "w1t": w1t, "w2t": w2t,
            "io4": io4, "io1": io1, "degr": dg, "dstt": dstt, "dln": dln,
            "srci": srci,
        })
    return in_maps


def kernel(x, W1, W2, src_idx, dst_idx, num_nodes):
    n_nodes = int(num_nodes)
    src_idx = np.asarray(src_idx).astype(np.int64)
    dst_idx = np.asarray(dst_idx).astype(np.int64)
    x = np.asarray(x, dtype=np.float32)
    sched = build_schedule(src_idx, dst_idx, n_nodes)
    nc = build_program(sched)
    in_maps = prepare_inputs(x, W1, W2, src_idx, dst_idx, n_nodes, sched)
    res = run_bass_kernel_spmd(nc, in_maps, list(range(NCORES)))
    SH = sched["SH"]
    out = np.concatenate([res.results[c]["out"][:SH] for c in range(NCORES)], axis=0)
    return out.astype(np.float32)


# revision 26
# speedup vs baseline: 1.2616x; 1.2616x over previous
"""DistSageConv trn2 kernel: out = x @ W2.T + (segment_sum(x[src], dst)/max(deg,1)) @ W1.T

Strategy (8 NeuronCores, SPMD single program):
- dst-shard: core c owns nodes [SH*c, SH*(c+1)) and all edges into them
  (dst_idx is sorted, so each core's edges are one contiguous slice).
- x replicated in each core's HBM as bf16; neighbor rows fetched with
  dma_gather (256B rows), 3 calls per (superblock, src-block) pass spread
  over the 4 SWDGE queues (~3.5k rows/call measured fastest: larger calls
  under-pipeline desc-gen vs transfer, smaller ones pay per-call fixed
  cost).
- src indices are int16-block-local (4 blocks of 25k rows); edges are
  packed [superblock s][src-block b][window-ordered slots] with a SHARED
  slot layout: per (s,b,w) capacity = max edge count over the 8 cores, so
  one SPMD program fits all cores (padding ~6% vs 25% for per-(s,b,w)
  chunk rounding).
- 128-edge chunks may span two adjacent 128-node dst windows; dstloc is
  stored window-pair-relative (0..255, exact in bf16; -1 pads).
- segment-sum on TensorE in bf16: per chunk, lhsT = gathered G [128 edges,
  128 feat], rhs = one-hot [128 edges, 128 window-cols]. PSUM accumulates
  agg_T [feat, node] for a whole superblock (20 windows, 5 banks).
- one-hots: first-window variant on DVE is_equal, 8 chunks per op (iota
  tile vs broadcast dstloc column); second-window variant (spanning
  chunks) on the Activation engine as relu(1 - (iota1 - d)^2) via Square
  then Relu with a per-partition bias, keeping DVE (the scarcest engine)
  off the ~20% spanning chunks.
- finals per window: z = agg_T.T @ W1T (PSUM), o = xT.T @ W2T (same PSUM
  tile), out = z / deg + o via one DVE scalar_tensor_tensor, DMA to HBM.
  The agg PSUM->SBUF staging is one batched Activation-engine copy per
  superblock.
- deg comes from host np.bincount over dst_idx: integer graph metadata
  derived from the index structure alone (same class as the slot layout
  computed for sharding); reciprocal + all float math stays on device.

The program is compiled per input graph (slot layout baked from the actual
indices, maxed over cores so all 8 run one SPMD program).

Measured on trn2 (8 cores, R=20001 rep differencing): 521us vs 1072us for
the fp32 per-(s,b,w)-chunk baseline. Decomposition: gather-only 345us,
one-hot-only 217us, matmul-only 92us, finals-only 40us.
"""

import numpy as np
import ml_dtypes

import concourse.bacc as bacc
import concourse.bass as bass
import concourse.mybir as mybir
import concourse.tile as tile
from concourse.bass_utils import run_bass_kernel_spmd

F32 = mybir.dt.float32
BF16 = mybir.dt.bfloat16
I16 = mybir.dt.int16

D = 128          # feature dim (both in and out)
NCORES = 8
WSZ = 128        # window = 128 dst nodes
SBW = 20         # windows per superblock (PSUM: 20*512B = 5 banks)
BLK = 25000      # src-block rows (int16-indexable)
NQ = 4           # SWDGE queues
CALL = 16384     # max gather slots per dma_gather call (one per pass)
GBUFS = 3        # gather buffer count


def build_schedule(src_idx, dst_idx, n_nodes, sbw=SBW):
    """Host-side integer bucketing. Returns shared slot layout + per-core
    edge streams. All cores share one program, so the per-(s,b,w) slot
    capacity is the max edge count over cores."""
    E = src_idx.shape[0]
    assert n_nodes % NCORES == 0
    SH = n_nodes // NCORES                      # nodes per core (12500)
    NW = (SH + WSZ - 1) // WSZ                  # windows per core (98)
    NSB = (NW + sbw - 1) // sbw                 # superblocks
    NB = (n_nodes + BLK - 1) // BLK             # src blocks (4)
    bounds = np.searchsorted(dst_idx, np.arange(NCORES + 1) * SH)

    # per-core group sizes: counts[c, s, b, w]
    counts = np.zeros((NCORES, NSB, NB, sbw), dtype=np.int64)
    per_core = []
    for c in range(NCORES):
        e0, e1 = bounds[c], bounds[c + 1]
        s_c, d_c = src_idx[e0:e1], dst_idx[e0:e1] - SH * c
        w = d_c // WSZ
        b = s_c // BLK
        sb = w // sbw
        # order edges by (sb, b, w); stable keeps dst order inside groups
        order = np.lexsort((w, b, sb))
        np.add.at(counts[c], (sb[order], b[order], w[order] % sbw), 1)
        per_core.append((s_c[order], d_c[order], w[order], b[order], sb[order]))

    cap = counts.max(axis=0)                    # [NSB, NB, SBW] slot capacity

    # shared slot layout per (s,b): windows packed back to back; chunks of
    # 128 slots; a chunk may span at most 2 adjacent windows (pad when a
    # third would creep in). Records, per pass: win_start[w], n_slots,
    # chunk w0/has_w1 lists.
    passes = {}
    for s in range(NSB):
        nwin = min(sbw, NW - s * sbw)
        for b in range(NB):
            pos = 0
            win_start = np.zeros(nwin, dtype=np.int64)
            chunk_w0 = {}                        # chunk idx -> first window
            for wl in range(nwin):
                if cap[s, b, wl] == 0:
                    win_start[wl] = pos
                    continue
                ck = pos // 128
                if pos % 128 != 0 and wl - chunk_w0[ck] >= 2:
                    pos = (ck + 1) * 128         # pad: keep span <= 2
                win_start[wl] = pos
                for k in range(pos // 128, (pos + cap[s, b, wl] + 127) // 128):
                    chunk_w0.setdefault(k, wl)
                pos += cap[s, b, wl]
            n_slots = ((pos + 127) // 128) * 128
            nch = pos and (pos + 127) // 128     # drop all-pad tail chunks
            w1flag = []
            for k in range(nch):
                w0 = chunk_w0[k]
                # second window = any window starting strictly inside chunk k
                has = any(
                    win_start[wl] > k * 128 and win_start[wl] < (k + 1) * 128
                    and cap[s, b, wl] > 0 and wl != w0
                    for wl in range(min(w0 + 1, nwin), min(w0 + 2, nwin))
                )
                w1flag.append(has)
            passes[(s, b)] = {
                "win_start": win_start, "n_slots": n_slots, "nch": nch,
                "chunk_w0": [chunk_w0.get(k, 0) for k in range(nch)],
                "has_w1": w1flag,
            }
            assert n_slots <= CALL, (s, b, n_slots)

    return {
        "SH": SH, "NW": NW, "NSB": NSB, "NB": NB, "SBW": sbw,
        "bounds": bounds, "counts": counts, "cap": cap, "passes": passes,
        "per_core": per_core,
    }


def build_host_arrays(sched, c):
    """srcloc int16 wrapped idx tile, dstloc bf16 tile for core c."""
    NW, NSB, NB = sched["NW"], sched["NSB"], sched["NB"]
    sbw = sched["SBW"]
    passes = sched["passes"]
    s_c, d_c, w_c, b_c, sb_c = sched["per_core"][c]
    counts_c = sched["counts"][c]

    NSLOT = sum(p["n_slots"] for p in passes.values())
    srcloc = np.zeros(NSLOT, dtype=np.int16)
    dstloc = np.full(NSLOT, -1.0, dtype=np.float32)

    pos = 0       # position in the (sb, b, w)-ordered per-core edge stream
    base = 0      # pass base slot
    for s in range(NSB):
        nwin = min(sbw, NW - s * sbw)
        for b in range(NB):
            p = passes[(s, b)]
            ws, w0s = p["win_start"], p["chunk_w0"]
            for wl in range(nwin):
                n = int(counts_c[s, b, wl])
                if n == 0:
                    continue
                slot = base + ws[wl]
                seg_s = s_c[pos : pos + n]
                seg_d = d_c[pos : pos + n]
                srcloc[slot : slot + n] = (seg_s - b * BLK).astype(np.int16)
                # dstloc relative to the owning chunk's first window
                sl = np.arange(slot, slot + n)
                ck = (sl - base) // 128
                w0 = np.array([w0s[k] for k in ck], dtype=np.int64)
                rel = (wl - w0) * WSZ + (seg_d - (s * sbw + wl) * WSZ)
                assert rel.min() >= 0 and rel.max() < 2 * WSZ, (s, b, wl)
                dstloc[slot : slot + n] = rel.astype(np.float32)
                pos += n
            base += p["n_slots"]
    assert pos == len(s_c) and base == NSLOT

    # wrap-16 idx layout replicated 8x: tile[p, j] = srcloc[j*16 + p%16]
    srci = np.tile(srcloc.reshape(NSLOT // 16, 16).T, (8, 1)).astype(np.int16)
    # dstloc tile [128, NCHUNK]: column k = slots [128k, 128k+128)
    dstt = dstloc.reshape(NSLOT // 128, 128).T.astype(ml_dtypes.bfloat16)
    return srci, dstt, NSLOT


def build_program(sched, rep_loop=False, no_compute=False, no_gather=False,
                  split=3, parts=("oh", "mm", "fin"), gbufs=GBUFS,
                  agg_bufs=None, fin_batch=True, oh1_act=True,
                  oh_mode="bcast", oh_swap=True, fin_direct=False,
                  zo_bufs=None, single_packet=False, ohb=8, delay_fin=None,
                  mm_delay=False, oh_bufs=None):
    SH, NW, NSB, NB = sched["SH"], sched["NW"], sched["NSB"], sched["NB"]
    SBWp = sched["SBW"]
    if agg_bufs is None:
        agg_bufs = 2 if SBWp <= 12 else 1
    if delay_fin is None:
        delay_fin = agg_bufs >= 2
    assert not (delay_fin and agg_bufs < 2)
    if zo_bufs is None:
        zo_bufs = 2 if agg_bufs >= 2 else 3
    passes = sched["passes"]
    NSLOT = sum(p["n_slots"] for p in passes.values())
    NCHUNK = sum(p["nch"] for p in passes.values())
    SHP = NW * WSZ                                  # padded shard rows (12544)
    NNODES = SH * NCORES

    OHB = ohb
    nc = bacc.Bacc("TRN2", target_bir_lowering=False, num_swdge_queues=NQ)
    xbf = nc.dram_tensor("xbf", [NNODES, D], BF16, kind="ExternalInput")
    xT = nc.dram_tensor("xT", [128, SHP], BF16, kind="ExternalInput")
    w1t = nc.dram_tensor("w1t", [128, D], BF16, kind="ExternalInput")
    w2t = nc.dram_tensor("w2t", [128, D], BF16, kind="ExternalInput")
    io4 = nc.dram_tensor("io4", [128, 8 * WSZ], BF16, kind="ExternalInput")
    io1 = nc.dram_tensor("io1", [128, WSZ], BF16, kind="ExternalInput")
    degr = nc.dram_tensor("degr", [128, NW], F32, kind="ExternalInput")
    dstt = nc.dram_tensor("dstt", [128, NCHUNK], BF16, kind="ExternalInput")
    dln = nc.dram_tensor("dln", [128, NCHUNK], F32, kind="ExternalInput")
    srci = nc.dram_tensor("srci", [128, NSLOT // 16], I16, kind="ExternalInput")
    if rep_loop:
        repsd = nc.dram_tensor("reps", [1, 1], mybir.dt.int32, kind="ExternalInput")
    outd = nc.dram_tensor("out", [SHP, D], F32, kind="ExternalOutput")

    with tile.TileContext(nc) as tc:
        with (
            tc.tile_pool(name="const", bufs=1) as constp,
            tc.tile_pool(name="gbuf", bufs=gbufs) as gpool,
            tc.tile_pool(name="oh", bufs=oh_bufs or (12 if mm_delay else 8)) as ohpool,
            tc.tile_pool(name="oh1", bufs=16 if mm_delay else 4) as oh1pool,
            tc.tile_pool(name="aggst", bufs=2 if fin_batch else 3) as aggstp,
            tc.tile_pool(name="outst", bufs=6) as outstp,
            tc.tile_pool(name="oself", bufs=3) as oselfp,
            tc.tile_pool(name="psA", bufs=agg_bufs, space="PSUM") as psagg,
            tc.tile_pool(name="psZ", bufs=zo_bufs, space="PSUM") as pszo,
        ):
            # ---- constants ----
            xT_t = constp.tile([128, SHP], BF16, tag="xT")
            w1_t = constp.tile([128, D], BF16, tag="w1")
            w2_t = constp.tile([128, D], BF16, tag="w2")
            io4_t = constp.tile([128, 8, WSZ], BF16, tag="io4")
            io1_t = constp.tile([128, 1, WSZ], BF16, tag="io1")
            dg_t = constp.tile([128, NW], F32, tag="degr")
            dl_t = constp.tile([128, NCHUNK], BF16, tag="dstt")
            dn_t = constp.tile([128, NCHUNK], F32, tag="dln")
            si_t = constp.tile([128, NSLOT // 16], I16, tag="srci")
            rc_t = constp.tile([128, NW], F32, tag="recip")
            zw_t = constp.tile([128, 128], BF16, tag="zw")
            zm_t = constp.tile([128, 512], BF16, tag="zm")
            nc.vector.memset(zw_t[:], 0.0)
            nc.vector.memset(zm_t[:], 0.0)
            nc.sync.dma_start(out=xT_t[:], in_=xT[:])
            nc.sync.dma_start(out=w1_t[:], in_=w1t[:])
            nc.sync.dma_start(out=w2_t[:], in_=w2t[:])
            nc.sync.dma_start(out=io4_t[:], in_=io4[:])
            nc.sync.dma_start(out=io1_t[:], in_=io1[:])
            nc.sync.dma_start(out=dg_t[:], in_=degr[:])
            nc.sync.dma_start(out=dl_t[:], in_=dstt[:])
            nc.sync.dma_start(out=dn_t[:], in_=dln[:])
            nc.sync.dma_start(out=si_t[:], in_=srci[:])
            nc.vector.reciprocal(out=rc_t[:], in_=dg_t[:])
            if rep_loop:
                reps_t = constp.tile([1, 1], mybir.dt.int32, tag="reps")
                nc.sync.dma_start(out=reps_t[:], in_=repsd[:])
                rv = nc.values_load(reps_t[0:1, 0:1])
                if isinstance(rv, tuple):
                    rv = rv[0]
                loop_cm = tc.For_i(0, rv, 1)
                loop_cm.__enter__()

            # ---- main loop ----
            pending_fin = None
            pending_mm = None
            call_i = 0            # global call counter (queue round robin)
            base = 0              # pass base slot
            ckbase = 0            # pass base chunk

            for s in range(NSB):
                wlo = s * SBWp
                nwin = min(SBWp, NW - wlo)
                agg = psagg.tile([128, SBWp * WSZ], F32, tag="agg")

                # clear each PSUM bank of agg: zeros matmul with start=True
                # (start clears has_written for the whole bank; writing the
                # full bank with zeros leaves every element 0 with bit set,
                # so all chunk matmuls below simply accumulate).
                ncols = SBWp * WSZ
                assert ncols % 512 == 0, SBWp
                for col0 in range(0, ncols, 512):
                    nc.tensor.matmul(
                        agg[:, col0 : col0 + 512],
                        lhsT=zw_t[:], rhs=zm_t[:],
                        start=True, stop=True,
                    )

                for b in range(NB):
                    p = passes[(s, b)]
                    nsl, nch = p["n_slots"], p["nch"]
                    if nch == 0:
                        continue
                    gt = None
                    if not no_gather:
                        gt = gpool.tile([128, CALL // 128, D], BF16, tag="g")
                        # split the pass gather across queues: concurrent
                        # transfers + finer-grained PE release
                        bnds = [(nch * j // split) for j in range(split + 1)]
                        for j in range(split):
                            c0, c1 = bnds[j], bnds[j + 1]
                            if c1 == c0:
                                continue
                            n = (c1 - c0) * 128
                            s0 = base + c0 * 128
                            nc.gpsimd.dma_gather(
                                gt[:, c0:c1],
                                xbf[b * BLK : min((b + 1) * BLK, NNODES)],
                                si_t[:, s0 // 16 : (s0 + n) // 16],
                                n, n, D,
                                single_packet=single_packet,
                                queue_num=call_i % NQ,
                            )
                            call_i += 1

                    do_oh = not no_compute and "oh" in parts
                    do_mm = not no_compute and "mm" in parts
                    # one-hots: variant 0 for every chunk (OHB per DVE op)
                    ohmap = {}
                    oh1map = {}
                    if do_oh and oh_mode == "stt":
                        # per-chunk: oh = is_equal(iota + (-d), 0) -- all
                        # non-scalar operands packed bf16 (2x-eligible)
                        for k in range(nch):
                            oht = ohpool.tile([128, WSZ], BF16, tag="oh")
                            nc.vector.scalar_tensor_tensor(
                                out=oht[:],
                                in0=io4_t[:, 0, :],
                                scalar=dn_t[:, ckbase + k : ckbase + k + 1],
                                in1=zw_t[:],
                                op0=mybir.AluOpType.add,
                                op1=mybir.AluOpType.is_equal,
                            )
                            ohmap[k] = oht
                    elif do_oh:
                        for q0 in range(0, nch, OHB):
                            nq4 = min(OHB, nch - q0)
                            oht = ohpool.tile([128, OHB, WSZ], BF16, tag="oh")
                            if oh_swap:
                                nc.vector.tensor_tensor(
                                    out=oht[:, :nq4, :],
                                    in0=dl_t[:, ckbase + q0 : ckbase + q0 + nq4]
                                        .to_broadcast((128, nq4, WSZ)),
                                    in1=io4_t[:, :nq4, :],
                                    op=mybir.AluOpType.is_equal,
                                )
                            else:
                                nc.vector.tensor_tensor(
                                    out=oht[:, :nq4, :],
                                    in0=io4_t[:, :nq4, :],
                                    in1=dl_t[:, ckbase + q0 : ckbase + q0 + nq4]
                                        .to_broadcast((128, nq4, WSZ)),
                                    op=mybir.AluOpType.is_equal,
                                )
                            ohmap[q0] = oht
                    if do_oh:
                        # variant 1 one-hots for window-spanning chunks
                        for k in range(nch):
                            if p["has_w1"][k]:
                                o1 = oh1pool.tile([128, 1, WSZ], BF16, tag="oh1")
                                if oh1_act:
                                    t1 = oh1pool.tile([128, 1, WSZ], BF16, tag="oh1t")
                                    nc.scalar.activation(
                                        out=t1[:, 0, :], in_=io1_t[:, 0, :],
                                        func=mybir.ActivationFunctionType.Square,
                                        bias=dn_t[:, ckbase + k : ckbase + k + 1],
                                    )
                                    nc.scalar.activation(
                                        out=o1[:, 0, :], in_=t1[:, 0, :],
                                        func=mybir.ActivationFunctionType.Relu,
                                        bias=1.0, scale=-1.0,
                                    )
                                else:
                                    nc.vector.tensor_tensor(
                                        out=o1[:],
                                        in0=io1_t[:],
                                        in1=dl_t[:, ckbase + k : ckbase + k + 1]
                                            .to_broadcast((128, 1, WSZ)),
                                        op=mybir.AluOpType.is_equal,
                                    )
                                oh1map[k] = o1

                    def emit_mms(p_, gt_, ohmap_, oh1map_):
                        for k in range(p_["nch"]):
                            w0 = p_["chunk_w0"][k]
                            gsl = zw_t[:] if no_gather else gt_[:, k]
                            if not do_oh:
                                oh0 = io4_t[:, 0, :]
                            elif oh_mode == "stt":
                                oh0 = ohmap_[k][:]
                            else:
                                oh0 = ohmap_[(k // OHB) * OHB][:, k % OHB, :]
                            nc.tensor.matmul(
                                agg[:, w0 * WSZ : (w0 + 1) * WSZ],
                                lhsT=gsl,
                                rhs=oh0,
                                start=False, stop=False,
                                skip_group_check=True,
                            )
                            if p_["has_w1"][k]:
                                oh1 = (oh1map_[k][:, 0, :]
                                       if do_oh else io4_t[:, 1, :])
                                nc.tensor.matmul(
                                    agg[:, (w0 + 1) * WSZ : (w0 + 2) * WSZ],
                                    lhsT=gsl,
                                    rhs=oh1,
                                    start=False, stop=False,
                                    skip_group_check=True,
                                )

                    if do_mm and not mm_delay:
                        emit_mms(p, gt, ohmap, oh1map)
                    elif do_mm:
                        # run PE one pass behind the gather frontier (within
                        # the superblock) so PE never stalls on the newest
                        # gather's completion
                        if pending_mm is not None:
                            emit_mms(*pending_mm)
                        pending_mm = (p, gt, ohmap, oh1map)
                    base += nsl
                    ckbase += nch

                # flush the delayed pass before finals touch agg
                if pending_mm is not None:
                    emit_mms(*pending_mm)
                    pending_mm = None

                # finals: emitted delayed by one superblock (delay_fin) so
                # the cross-engine finals chain never blocks the next
                # superblock's one-hot stream on DVE.
                if no_compute or "fin" not in parts:
                    continue

                def emit_finals(agg_, wlo_, nwin_):
                    if fin_batch:
                        agbig = aggstp.tile([128, SBWp * WSZ], BF16, tag="aggst")
                        nc.scalar.copy(out=agbig[:, : nwin_ * WSZ],
                                       in_=agg_[:, : nwin_ * WSZ])
                    for wl in range(nwin_):
                        w = wlo_ + wl
                        if fin_batch:
                            ag = agbig[:, wl * WSZ : (wl + 1) * WSZ]
                        else:
                            agt = aggstp.tile([128, WSZ], BF16, tag="aggst")
                            nc.scalar.copy(out=agt[:],
                                           in_=agg_[:, wl * WSZ : (wl + 1) * WSZ])
                            ag = agt[:]
                        zo = pszo.tile([128, 2 * WSZ], F32, tag="zo")
                        nc.tensor.matmul(zo[:, 0:WSZ], lhsT=ag, rhs=w1_t[:],
                                         start=True, stop=True)
                        nc.tensor.matmul(zo[:, WSZ : 2 * WSZ],
                                         lhsT=xT_t[:, w * WSZ : (w + 1) * WSZ],
                                         rhs=w2_t[:], start=True, stop=True)
                        ot = outstp.tile([128, D], F32, tag="outst")
                        if fin_direct:
                            in1ap = zo[:, WSZ : 2 * WSZ]
                        else:
                            o_sb = oselfp.tile([128, D], BF16, tag="oself")
                            nc.scalar.copy(out=o_sb[:], in_=zo[:, WSZ : 2 * WSZ])
                            in1ap = o_sb[:]
                        nc.vector.scalar_tensor_tensor(
                            out=ot[:], in0=zo[:, 0:WSZ],
                            scalar=rc_t[:, w : w + 1],
                            in1=in1ap,
                            op0=mybir.AluOpType.mult,
                            op1=mybir.AluOpType.add,
                        )
                        nc.sync.dma_start(
                            out=outd[w * WSZ : (w + 1) * WSZ], in_=ot[:]
                        )

                if delay_fin:
                    if pending_fin is not None:
                        emit_finals(*pending_fin)
                    pending_fin = (agg, wlo, nwin)
                else:
                    emit_finals(agg, wlo, nwin)

            if not no_compute and "fin" in parts and pending_fin is not None:
                emit_finals(*pending_fin)

            if rep_loop:
                loop_cm.__exit__(None, None, None)
    nc.compile()
    return nc


def prepare_inputs(x, W1, W2, src_idx, dst_idx, n_nodes, sched):
    SH, NW = sched["SH"], sched["NW"]
    SHP = NW * WSZ
    deg = np.bincount(dst_idx, minlength=n_nodes).astype(np.float32)
    deg = np.maximum(deg, 1.0)
    x = np.ascontiguousarray(x, dtype=np.float32)
    xb = x.astype(ml_dtypes.bfloat16)
    w1t = np.ascontiguousarray(W1.T).astype(ml_dtypes.bfloat16)
    w2t = np.ascontiguousarray(W2.T).astype(ml_dtypes.bfloat16)
    io4 = np.tile(np.arange(WSZ, dtype=np.float32), (128, 8)).astype(ml_dtypes.bfloat16)
    io1 = np.tile(WSZ + np.arange(WSZ, dtype=np.float32), (128, 1)).astype(ml_dtypes.bfloat16)

    in_maps = []
    for c in range(NCORES):
        srci, dstt, _ = build_host_arrays(sched, c)
        dln = -dstt.astype(np.float32)
        xs = xb[SH * c : SH * (c + 1)]
        xT = np.zeros((128, SHP), dtype=ml_dtypes.bfloat16)
        xT[:, :SH] = xs.T
        dgflat = deg[SH * c : SH * (c + 1)]
        dgp = np.ones(SHP, dtype=np.float32)
        dgp[:SH] = dgflat
        dg = dgp.reshape(NW, WSZ).T.copy()
        in_maps.append({
            "xbf": xb, "xT": xT, "w1t": w1t, "w2t": w2t,
            "io4": io4, "io1": io1, "degr": dg, "dstt": dstt, "dln": dln,
            "srci": srci,
        })
    return in_maps


def kernel(x, W1, W2, src_idx, dst_idx, num_nodes):
    n_nodes = int(num_nodes)
    src_idx = np.asarray(src_idx).astype(np.int64)
    dst_idx = np.asarray(dst_idx).astype(np.int64)
    x = np.asarray(x, dtype=np.float32)
    sched = build_schedule(src_idx, dst_idx, n_nodes)
    nc = build_program(sched)
    in_maps = prepare_inputs(x, W1, W2, src_idx, dst_idx, n_nodes, sched)
    res = run_bass_kernel_spmd(nc, in_maps, list(range(NCORES)))
    SH = sched["SH"]
    out = np.concatenate([res.results[c]["out"][:SH] for c in range(NCORES)], axis=0)
    return out.astype(np.float32)
